# revision 16
# baseline (speedup 1.0000x reference)
"""Causal self-attention (B=2, T=2048, d_model=1024, H=16) on 8 TRN2 NeuronCores.

Sharding: core c handles batch b = c//4 and head group g = c%4 (heads 4g..4g+3).
Each core computes QKV projection for its heads, causal attention, and a partial
output projection y_partial = attn_out @ Wo[g*256:(g+1)*256, :]. The host sums
the 4 partials per batch (the tensor-parallel all-reduce, done on host).

v2 (bf16 pipeline):
  - all PE operands bf16 (host-cast weights/x), f32 PSUM accumulation.
    FWL fast weight load + warm-clock matmuls.
  - softmax normalization batched at end of attention: denominators DVE-copied
    to 4 contiguous rows, one reciprocal_approx_fast, DMA-scatter to aligned
    partitions {0,32,64,96}, then per (pair, ic) a K=1 PE matmul broadcasts
    1/sum across 64 partitions into PSUM and one DVE mult normalizes both
    heads of the pair at once.  (The old per-pair reciprocal + 256KB broadcast
    DMA blocked the DVE FIFO ~14.5us per pair and let the PE clock re-throttle.)
  - xT prefetched one rep ahead (bufs=2), qT/kT double-buffered so next rep's
    QKV overlaps current rep's attention.

Layouts on device (per core):
  xT  [1024, 2048] bf16 = x[b].T          (contraction dim on partitions)
  qT/kT [128, 2, 2048] bf16               (two heads packed per 128 partitions,
                                           head dim 64 on partitions)
  S^T tiles [128 keys, <=512 queries] f32 PSUM; exp on ACT -> et bf16;
  V' = [V | 1] so the PV matmul also produces per-query denominators.
"""
import sys

sys.path.insert(0, "/opt/trn_rl_repo")

import numpy as np

B, T, C = 2, 2048, 1024
NH_TOT = 16
HD = 64
NH = 4          # heads per core
CO = NH * HD    # 256 channels per core
NCORES = 8
SCALE = 1.0 / 32.0  # d_model ** -0.5

_compiled = None


def _build(nrep=1, trace_sim=False):
    import concourse.bass as bass  # noqa: F401
    import concourse.mybir as mybir
    import concourse.tile as tile
    from concourse import bacc

    F32 = mybir.dt.float32
    F32R = mybir.dt.float32r
    BF16 = mybir.dt.bfloat16
    MULT = mybir.AluOpType.mult
    EXP = mybir.ActivationFunctionType.Exp

    nc = bacc.Bacc("TRN2", target_bir_lowering=False)

    xT = nc.declare_dram_parameter("xT", [C, T], BF16, isOutput=False)
    wq = nc.declare_dram_parameter("wq", [C, CO], BF16, isOutput=False)
    wk = nc.declare_dram_parameter("wk", [C, CO], BF16, isOutput=False)
    wv = nc.declare_dram_parameter("wv", [C, CO], BF16, isOutput=False)
    wo = nc.declare_dram_parameter("wo", [CO, C], BF16, isOutput=False)
    mask = nc.declare_dram_parameter("mask", [128, 128], BF16, isOutput=False)
    y = nc.declare_dram_parameter("y", [T, C], F32, isOutput=True)

    xT_t = xT.rearrange("(o p) t -> p o t", p=128)   # [128, 8, 2048]
    wq_t = wq.rearrange("(o p) m -> p o m", p=128)   # [128, 8, 256]
    wk_t = wk.rearrange("(o p) m -> p o m", p=128)
    wv_t = wv.rearrange("(o p) m -> p o m", p=128)
    wo_t = wo.rearrange("(o p) m -> p o m", p=128)   # [128, 2, 1024]

    with tile.TileContext(nc, trace_sim=trace_sim) as tc:
        with (
            nc.allow_low_precision(reason="bf16 matmul pipeline"),
            tc.tile_pool(name="wpool", bufs=1) as wpool,
            tc.tile_pool(name="qkvpool", bufs=1) as qkvpool,
            tc.tile_pool(name="xpool", bufs=2) as xpool,
            tc.tile_pool(name="etpool", bufs=6) as etpool,
            tc.tile_pool(name="ypool", bufs=2) as ypool,
            tc.tile_pool(name="psa", bufs=2, space="PSUM") as psa,
        ):
            wq_sb = wpool.tile([128, 8, CO], BF16, tag="wq")
            wk_sb = wpool.tile([128, 8, CO], BF16, tag="wk")
            wv_sb = wpool.tile([128, 8, CO], BF16, tag="wv")
            wo_sb = wpool.tile([128, 2, C], BF16, tag="wo")
            mask_sb = wpool.tile([128, 128], BF16, tag="mask")
            ones_sb = wpool.tile([128, HD], BF16, tag="ones")
            nc.sync.dma_start(wq_sb[:], wq_t[:])
            nc.sync.dma_start(wk_sb[:], wk_t[:])
            nc.sync.dma_start(wv_sb[:], wv_t[:])
            nc.sync.dma_start(wo_sb[:], wo_t[:])
            nc.sync.dma_start(mask_sb[:], mask[:])
            nc.vector.memset(ones_sb[:], 1.0)

            # V' ones columns (col HD of each head) are set once per slot
            # below, right after the first allocation of each vp buffer.
            oT_sb = qkvpool.tile([128, 2, T], BF16, tag="oT")
            sums_sb = qkvpool.tile([128, T], BF16, tag="sums")

            def load_x(xt):
                for th in range(2):
                    for kc in range(8):
                        nc.sync.dma_start(
                            xt[:, kc, th * 1024:(th + 1) * 1024],
                            xT_t[:, kc, th * 1024:(th + 1) * 1024],
                        )

            def qkv_phase(xT_sb):
                """QKV projection; returns (qT, kT, vp) tiles for the rep."""
                qT_sb = qkvpool.tile([128, 2, T], BF16, tag="qT", bufs=2)
                kT_sb = qkvpool.tile([128, 2, T], BF16, tag="kT", bufs=2)
                vp_sb = qkvpool.tile([128, 16, NH, HD + 1], BF16, tag="vp",
                                     bufs=2)
                nc.vector.memset(vp_sb[:, :, :, HD], 1.0)

                # qT/kT: [c_out pair on partitions, t free]
                for t8 in range(2):
                    for w_sb, dst in ((wq_sb, qT_sb), (wk_sb, kT_sb)):
                        for m in range(2):
                            pq = psa.tile([128, 1024], F32, tag="s", name="pq")
                            for half in range(2):
                                t0c = t8 * 1024 + half * 512
                                for kc in range(8):
                                    nc.tensor.matmul(
                                        pq[:, half * 512:(half + 1) * 512],
                                        w_sb[:, kc, m * 128:(m + 1) * 128],
                                        xT_sb[:, kc, t0c:t0c + 512],
                                        start=(kc == 0),
                                        stop=(kc == 7),
                                    )
                            nc.vector.tensor_copy(
                                dst[:, m, t8 * 1024:(t8 + 1) * 1024], pq[:]
                            )

                # V in [t on partitions, head channels] layout
                for tb in range(16):
                    pv = psa.tile([128, 1024], F32, tag="s", name="pv")
                    for kc in range(8):
                        nc.tensor.matmul(
                            pv[:, 0:CO],
                            xT_sb[:, kc, tb * 128:(tb + 1) * 128],
                            wv_sb[:, kc, :],
                            start=(kc == 0),
                            stop=(kc == 7),
                        )
                    nc.vector.tensor_copy(
                        vp_sb[:, tb, :, 0:HD],
                        pv[:, 0:CO].rearrange("p (h d) -> p h d", h=NH),
                    )
                return qT_sb, kT_sb, vp_sb

            def att_phase(qT_sb, kT_sb, vp_sb):
                for pair in range(NH // 2):
                    heads = (2 * pair, 2 * pair + 1)

                    for ic in range(2):
                        i_base = 1024 * ic
                        jb_last = 8 * ic + 7
                        pos = [
                            psa.tile([65, 1024], F32, tag=f"o{hi}",
                                     bufs=1, name=f"po{hi}")
                            for hi in range(2)
                        ]

                        def emit_s(h, jb):
                            po2, mo2 = h % 2, h // 2
                            i0 = max(i_base, 128 * jb)
                            k_h = kT_sb[64 * po2:64 * po2 + 64, mo2, :]
                            q_h = qT_sb[64 * po2:64 * po2 + 64, mo2, :]
                            ps_s = psa.tile([128, 1024], F32, tag="s",
                                            bufs=2, name="ps_s")
                            off = i0 - i_base
                            while off < 1024:
                                w = min(512 - off % 512, 1024 - off)
                                nc.tensor.matmul(
                                    ps_s[:, off:off + w],
                                    k_h[:, jb * 128:(jb + 1) * 128],
                                    q_h[:, i_base + off:i_base + off + w],
                                    start=True,
                                    stop=True,
                                )
                                off += w
                            et = etpool.tile([128, 1024], BF16, tag="et",
                                             name="et")
                            o0 = i0 - i_base
                            nc.scalar.activation(
                                et[:, o0:1024], ps_s[:, o0:1024], EXP,
                                scale=SCALE,
                            )
                            if 128 * jb >= i_base:
                                nc.vector.tensor_tensor(
                                    et[:, o0:o0 + 128], et[:, o0:o0 + 128],
                                    mask_sb[:], MULT,
                                )
                            return et, i0

                        def emit_pv(hi, jb, et, i0):
                            off = i0 - i_base
                            while off < 1024:
                                w = min(512 - off % 512, 1024 - off)
                                nc.tensor.matmul(
                                    pos[hi][:, off:off + w],
                                    vp_sb[:, jb, heads[hi], :],
                                    et[:, off:off + w],
                                    start=(jb == 0),
                                    stop=(jb == jb_last),
                                )
                                off += w

                        pending = [emit_s(h, 0) for h in heads]
                        for jb in range(jb_last + 1):
                            nxt = None
                            if jb < jb_last:
                                nxt = [emit_s(h, jb + 1) for h in heads]
                            for hi in range(2):
                                emit_pv(hi, jb, *pending[hi])
                            if nxt is not None:
                                pending = nxt

                        # stage unnormalized O^T and the denominators
                        isl = slice(i_base, i_base + 1024)
                        for hi, h in enumerate(heads):
                            po2 = h % 2
                            dst = oT_sb[64 * po2:64 * po2 + 64, pair, isl]
                            nc.vector.tensor_copy(dst, pos[hi][0:64, :])
                            nc.vector.tensor_copy(
                                sums_sb[32 * h:32 * h + 1, isl],
                                pos[hi][64:65, :],
                            )

            def norm_phase():
                # broadcast sums across 64 partitions via K=1 matmuls, then
                # full-lane reciprocal on the [128,1024] PSUM tile.
                for pair in range(NH // 2):
                    for ic in range(2):
                        isl = slice(1024 * ic, 1024 * ic + 1024)
                        bc = psa.tile([128, 1024], F32, tag="s", name="bc")
                        for hi in range(2):
                            h = 2 * pair + hi
                            for half in range(2):
                                csl = slice(1024 * ic + 512 * half,
                                            1024 * ic + 512 * half + 512)
                                nc.tensor.matmul(
                                    bc[64 * hi:64 * hi + 64,
                                       512 * half:512 * half + 512],
                                    ones_sb[32 * h:32 * h + 1, :],
                                    sums_sb[32 * h:32 * h + 1, csl],
                                    start=True,
                                    stop=True,
                                    tile_position=(32 * h, 64 * hi),
                                )
                        rb = qkvpool.tile([128, 1024], F32, tag="rb", bufs=2)
                        nc.vector.reciprocal_approx_fast(rb[:], bc[:])
                        o_sl = oT_sb[:, pair, isl]
                        nc.vector.tensor_tensor(o_sl, o_sl, rb[:], MULT)

            def proj_phase():
                # two t-blocks share one SBUF tile so each y DMA moves 1 MiB
                for tb2 in range(8):
                    y2 = ypool.tile([128, 2, C], F32, tag="yt", name="y2")
                    for sub in range(2):
                        tb = 2 * tb2 + sub
                        py = psa.tile([128, 1024], F32, tag="s", name="py")
                        for nk in range(2):
                            for cp in range(2):
                                nc.tensor.matmul(
                                    py[:, nk * 512:(nk + 1) * 512],
                                    oT_sb[:, cp, tb * 128:(tb + 1) * 128],
                                    wo_sb[:, cp, nk * 512:(nk + 1) * 512],
                                    start=(cp == 0),
                                    stop=(cp == 1),
                                )
                        dst = y2[:, sub, :]
                        if sub == 0:
                            nc.scalar.copy(dst, py[:])
                        else:
                            nc.vector.tensor_copy(dst, py[:])
                    nc.gpsimd.dma_start(
                        y[tb2 * 256:(tb2 + 1) * 256, :].rearrange(
                            "(b p) c -> p b c", p=128
                        ),
                        y2[:],
                    )

            # ---- software-pipelined rep loop ----
            # emission order per rep: [xT(r+1) load] att(r) [qkv(r+1)]
            # norm(r) proj(r) -- next rep's QKV fills the PE during this
            # rep's normalize/proj serialization, and xT transfers during
            # attention.
            xt_cur = xpool.tile([128, 8, T], BF16, tag="xT", bufs=2)
            load_x(xt_cur)
            cur = qkv_phase(xt_cur)
            for _rep in range(nrep):
                if _rep + 1 < nrep:
                    xt_nxt = xpool.tile([128, 8, T], BF16, tag="xT", bufs=2)
                    load_x(xt_nxt)
                att_phase(*cur)
                norm_phase()
                if _rep + 1 < nrep:
                    cur = qkv_phase(xt_nxt)
                proj_phase()

    nc.compile()
    return nc


def _get_nc():
    global _compiled
    if _compiled is None:
        _compiled = _build()
    return _compiled


class _Runner:
    """Compiled PJRT executor for the SPMD kernel, reusable across calls."""

    def __init__(self, nc):
        import jax
        import concourse.mybir as mybir
        from concourse import bass2jax
        from jax.experimental.shard_map import shard_map
        from jax.sharding import Mesh, PartitionSpec

        self.jax = jax
        self.nc = nc
        bass2jax.install_neuronx_cc_hook()

        partition_name = (
            nc.partition_id_tensor.name if nc.partition_id_tensor else None
        )
        in_names, out_names, out_avals, zero_outs = [], [], [], []
        for alloc in nc.m.functions[0].allocations:
            if not isinstance(alloc, mybir.MemoryLocationSet):
                continue
            name = alloc.memorylocations[0].name
            if alloc.kind == "ExternalInput":
                if name != partition_name:
                    in_names.append(name)
            elif alloc.kind == "ExternalOutput":
                out_names.append(name)
                shape = tuple(alloc.tensor_shape)
                dtype = mybir.dt.np(alloc.dtype)
                out_avals.append(jax.core.ShapedArray(shape, dtype))
                zero_outs.append(np.zeros(shape, dtype))
        self.in_names = in_names
        self.out_names = out_names
        self.out_avals = out_avals
        self.zero_outs = zero_outs
        all_names = tuple(in_names + out_names)

        if partition_name is not None:
            all_names = all_names + (partition_name,)

        def _body(*args):
            operands = list(args)
            if partition_name is not None:
                operands.append(bass2jax.partition_id_tensor())
            outs = bass2jax._bass_exec_p.bind(
                *operands,
                out_avals=tuple(out_avals),
                in_names=all_names,
                out_names=tuple(out_names),
                lowering_input_output_aliases=(),
                sim_require_finite=True,
                sim_require_nnan=True,
                nc=nc,
            )
            return tuple(outs)

        devices = jax.devices()[:NCORES]
        assert len(devices) == NCORES
        mesh = Mesh(np.asarray(devices), ("core",))
        self._sharding = jax.sharding.NamedSharding(mesh, PartitionSpec("core"))
        n_args = len(in_names) + len(out_names)
        self.fn = jax.jit(
            shard_map(
                _body,
                mesh=mesh,
                in_specs=(PartitionSpec("core"),) * n_args,
                out_specs=(PartitionSpec("core"),) * len(out_names),
                check_rep=False,
            ),
            keep_unused=True,
        )

    def device_args(self, in_maps):
        args = [
            np.concatenate([np.asarray(m[name]) for m in in_maps], axis=0)
            for name in self.in_names
        ]
        args += [
            np.zeros((NCORES * z.shape[0], *z.shape[1:]), z.dtype)
            for z in self.zero_outs
        ]
        return [self.jax.device_put(a, self._sharding) for a in args]

    def run_device(self, dev_args):
        return self.fn(*dev_args)

    def run(self, in_maps):
        out_arrs = self.fn(*self.device_args(in_maps))
        return [
            {
                name: np.asarray(out_arrs[i]).reshape(
                    NCORES, *self.out_avals[i].shape
                )[c]
                for i, name in enumerate(self.out_names)
            }
            for c in range(NCORES)
        ]


_runner = None


def _get_runner():
    global _runner
    if _runner is None:
        _runner = _Runner(_get_nc())
    return _runner


def make_in_maps(x, Wqkv, Wo):
    import ml_dtypes

    bf16 = ml_dtypes.bfloat16
    x = np.asarray(x, dtype=np.float32)
    Wqkv = np.asarray(Wqkv, dtype=np.float32)
    Wo = np.asarray(Wo, dtype=np.float32)
    mask = np.triu(np.ones((128, 128), dtype=np.float32)).astype(bf16)
    in_maps = []
    for c in range(NCORES):
        b, g = c // 4, c % 4
        in_maps.append({
            "xT": np.ascontiguousarray(x[b].T).astype(bf16),
            "wq": np.ascontiguousarray(
                Wqkv[:, g * CO:(g + 1) * CO]).astype(bf16),
            "wk": np.ascontiguousarray(
                Wqkv[:, C + g * CO:C + (g + 1) * CO]).astype(bf16),
            "wv": np.ascontiguousarray(
                Wqkv[:, 2 * C + g * CO:2 * C + (g + 1) * CO]).astype(bf16),
            "wo": np.ascontiguousarray(Wo[g * CO:(g + 1) * CO, :]).astype(bf16),
            "mask": mask,
        })
    return in_maps


def gather_output(results):
    y = np.zeros((B, T, C), dtype=np.float32)
    for c in range(NCORES):
        y[c // 4] += results[c]["y"]
    return y


def kernel(x, Wqkv, Wo):
    runner = _get_runner()
    in_maps = make_in_maps(x, Wqkv, Wo)
    return gather_output(runner.run(in_maps))


# revision 17
# speedup vs baseline: 1.0588x; 1.0588x over previous
"""Causal self-attention (B=2, T=2048, d_model=1024, H=16) on 8 TRN2 NeuronCores.

Sharding: core c handles batch b = c//4 and head group g = c%4 (heads 4g..4g+3).
Each core computes QKV projection for its heads, causal attention, and a partial
output projection y_partial = attn_out @ Wo[g*256:(g+1)*256, :]. The host sums
the 4 partials per batch (the tensor-parallel all-reduce, done on host).

v2 (bf16 pipeline):
  - all PE operands bf16 (host-cast weights/x), f32 PSUM accumulation.
    FWL fast weight load + warm-clock matmuls.
  - softmax normalization batched at end of attention: denominators DVE-copied
    to 4 contiguous rows, one reciprocal_approx_fast, DMA-scatter to aligned
    partitions {0,32,64,96}, then per (pair, ic) a K=1 PE matmul broadcasts
    1/sum across 64 partitions into PSUM and one DVE mult normalizes both
    heads of the pair at once.  (The old per-pair reciprocal + 256KB broadcast
    DMA blocked the DVE FIFO ~14.5us per pair and let the PE clock re-throttle.)
  - xT prefetched one rep ahead (bufs=2), qT/kT double-buffered so next rep's
    QKV overlaps current rep's attention.

Layouts on device (per core):
  xT  [1024, 2048] bf16 = x[b].T          (contraction dim on partitions)
  qT/kT [128, 2, 2048] bf16               (two heads packed per 128 partitions,
                                           head dim 64 on partitions)
  S^T tiles [128 keys, <=512 queries] f32 PSUM; exp on ACT -> et bf16;
  V' = [V | 1] so the PV matmul also produces per-query denominators.
"""
import sys

sys.path.insert(0, "/opt/trn_rl_repo")

import numpy as np

B, T, C = 2, 2048, 1024
NH_TOT = 16
HD = 64
NH = 4          # heads per core
CO = NH * HD    # 256 channels per core
NCORES = 8
SCALE = 1.0 / 32.0  # d_model ** -0.5

_compiled = None


def _build(nrep=1, trace_sim=False):
    import concourse.bass as bass  # noqa: F401
    import concourse.mybir as mybir
    import concourse.tile as tile
    from concourse import bacc

    F32 = mybir.dt.float32
    F32R = mybir.dt.float32r
    BF16 = mybir.dt.bfloat16
    MULT = mybir.AluOpType.mult
    EXP = mybir.ActivationFunctionType.Exp

    nc = bacc.Bacc("TRN2", target_bir_lowering=False)

    xT = nc.declare_dram_parameter("xT", [C, T], BF16, isOutput=False)
    wq = nc.declare_dram_parameter("wq", [C, CO], BF16, isOutput=False)
    wk = nc.declare_dram_parameter("wk", [C, CO], BF16, isOutput=False)
    wv = nc.declare_dram_parameter("wv", [C, CO], BF16, isOutput=False)
    wo = nc.declare_dram_parameter("wo", [CO, C], BF16, isOutput=False)
    mask = nc.declare_dram_parameter("mask", [128, 128], BF16, isOutput=False)
    y = nc.declare_dram_parameter("y", [T, C], F32, isOutput=True)

    xT_t = xT.rearrange("(o p) t -> p o t", p=128)   # [128, 8, 2048]
    wq_t = wq.rearrange("(o p) m -> p o m", p=128)   # [128, 8, 256]
    wk_t = wk.rearrange("(o p) m -> p o m", p=128)
    wv_t = wv.rearrange("(o p) m -> p o m", p=128)
    wo_t = wo.rearrange("(o p) m -> p o m", p=128)   # [128, 2, 1024]

    with tile.TileContext(nc, trace_sim=trace_sim) as tc:
        with (
            nc.allow_low_precision(reason="bf16 matmul pipeline"),
            tc.tile_pool(name="wpool", bufs=1) as wpool,
            tc.tile_pool(name="qkvpool", bufs=1) as qkvpool,
            tc.tile_pool(name="xpool", bufs=2) as xpool,
            tc.tile_pool(name="etpool", bufs=6) as etpool,
            tc.tile_pool(name="ypool", bufs=2) as ypool,
            tc.tile_pool(name="psa", bufs=2, space="PSUM") as psa,
        ):
            wq_sb = wpool.tile([128, 8, CO], BF16, tag="wq")
            wk_sb = wpool.tile([128, 8, CO], BF16, tag="wk")
            wv_sb = wpool.tile([128, 8, CO], BF16, tag="wv")
            wo_sb = wpool.tile([128, 2, C], BF16, tag="wo")
            mask_sb = wpool.tile([128, 128], BF16, tag="mask")
            ones_sb = wpool.tile([128, HD], BF16, tag="ones")
            nc.sync.dma_start(wq_sb[:], wq_t[:])
            nc.sync.dma_start(wk_sb[:], wk_t[:])
            nc.sync.dma_start(wv_sb[:], wv_t[:])
            nc.sync.dma_start(wo_sb[:], wo_t[:])
            nc.sync.dma_start(mask_sb[:], mask[:])
            nc.vector.memset(ones_sb[:], 1.0)

            # V' ones columns (col HD of each head) are set once per slot
            # below, right after the first allocation of each vp buffer.
            oT_sb = qkvpool.tile([128, 2, T], BF16, tag="oT")
            sums_sb = qkvpool.tile([128, T], BF16, tag="sums")

            def load_x(xt):
                for th in range(2):
                    for kc in range(8):
                        nc.sync.dma_start(
                            xt[:, kc, th * 1024:(th + 1) * 1024],
                            xT_t[:, kc, th * 1024:(th + 1) * 1024],
                        )

            def qkv_phase(xT_sb):
                """QKV projection; returns (qT, kT, vp) tiles for the rep."""
                qT_sb = qkvpool.tile([128, 2, T], BF16, tag="qT", bufs=2)
                kT_sb = qkvpool.tile([128, 2, T], BF16, tag="kT", bufs=2)
                vp_sb = qkvpool.tile([128, 16, NH, HD + 1], BF16, tag="vp",
                                     bufs=2)
                nc.vector.memset(vp_sb[:, :, :, HD], 1.0)

                # qT/kT: [c_out pair on partitions, t free]
                for t8 in range(2):
                    for w_sb, dst in ((wq_sb, qT_sb), (wk_sb, kT_sb)):
                        for m in range(2):
                            pq = psa.tile([128, 1024], F32, tag="s", name="pq")
                            for half in range(2):
                                t0c = t8 * 1024 + half * 512
                                for kc in range(8):
                                    nc.tensor.matmul(
                                        pq[:, half * 512:(half + 1) * 512],
                                        w_sb[:, kc, m * 128:(m + 1) * 128],
                                        xT_sb[:, kc, t0c:t0c + 512],
                                        start=(kc == 0),
                                        stop=(kc == 7),
                                    )
                            nc.vector.tensor_copy(
                                dst[:, m, t8 * 1024:(t8 + 1) * 1024], pq[:]
                            )

                # V in [t on partitions, head channels] layout
                for tb in range(16):
                    pv = psa.tile([128, 1024], F32, tag="s", name="pv")
                    for kc in range(8):
                        nc.tensor.matmul(
                            pv[:, 0:CO],
                            xT_sb[:, kc, tb * 128:(tb + 1) * 128],
                            wv_sb[:, kc, :],
                            start=(kc == 0),
                            stop=(kc == 7),
                        )
                    nc.vector.tensor_copy(
                        vp_sb[:, tb, :, 0:HD],
                        pv[:, 0:CO].rearrange("p (h d) -> p h d", h=NH),
                    )
                return qT_sb, kT_sb, vp_sb

            def att_phase(qT_sb, kT_sb, vp_sb):
                for pair in range(NH // 2):
                    heads = (2 * pair, 2 * pair + 1)

                    for ic in range(2):
                        i_base = 1024 * ic
                        jb_last = 8 * ic + 7
                        pos = [
                            psa.tile([65, 1024], F32, tag=f"o{hi}",
                                     bufs=1, name=f"po{hi}")
                            for hi in range(2)
                        ]

                        def emit_s(h, jb):
                            po2, mo2 = h % 2, h // 2
                            i0 = max(i_base, 128 * jb)
                            k_h = kT_sb[64 * po2:64 * po2 + 64, mo2, :]
                            q_h = qT_sb[64 * po2:64 * po2 + 64, mo2, :]
                            ps_s = psa.tile([128, 1024], F32, tag="s",
                                            bufs=2, name="ps_s")
                            off = i0 - i_base
                            while off < 1024:
                                w = min(512 - off % 512, 1024 - off)
                                nc.tensor.matmul(
                                    ps_s[:, off:off + w],
                                    k_h[:, jb * 128:(jb + 1) * 128],
                                    q_h[:, i_base + off:i_base + off + w],
                                    start=True,
                                    stop=True,
                                )
                                off += w
                            et = etpool.tile([128, 1024], BF16, tag="et",
                                             name="et")
                            o0 = i0 - i_base
                            nc.scalar.activation(
                                et[:, o0:1024], ps_s[:, o0:1024], EXP,
                                scale=SCALE,
                            )
                            if 128 * jb >= i_base:
                                nc.vector.tensor_tensor(
                                    et[:, o0:o0 + 128], et[:, o0:o0 + 128],
                                    mask_sb[:], MULT,
                                )
                            return et, i0

                        def emit_pv(hi, jb, et, i0):
                            off = i0 - i_base
                            while off < 1024:
                                w = min(512 - off % 512, 1024 - off)
                                nc.tensor.matmul(
                                    pos[hi][:, off:off + w],
                                    vp_sb[:, jb, heads[hi], :],
                                    et[:, off:off + w],
                                    start=(jb == 0),
                                    stop=(jb == jb_last),
                                )
                                off += w

                        pending = [emit_s(h, 0) for h in heads]
                        for jb in range(jb_last + 1):
                            nxt = None
                            if jb < jb_last:
                                nxt = [emit_s(h, jb + 1) for h in heads]
                            for hi in range(2):
                                emit_pv(hi, jb, *pending[hi])
                            if nxt is not None:
                                pending = nxt

                        # stage unnormalized O^T and the denominators
                        isl = slice(i_base, i_base + 1024)
                        for hi, h in enumerate(heads):
                            po2 = h % 2
                            dst = oT_sb[64 * po2:64 * po2 + 64, pair, isl]
                            nc.vector.tensor_copy(dst, pos[hi][0:64, :])
                            nc.vector.tensor_copy(
                                sums_sb[32 * h:32 * h + 1, isl],
                                pos[hi][64:65, :],
                            )

            def norm_phase():
                # broadcast sums across 64 partitions via K=1 matmuls, then
                # full-lane reciprocal on the [128,1024] PSUM tile.
                for pair in range(NH // 2):
                    for ic in range(2):
                        isl = slice(1024 * ic, 1024 * ic + 1024)
                        bc = psa.tile([128, 1024], F32, tag="s", name="bc")
                        for hi in range(2):
                            h = 2 * pair + hi
                            for half in range(2):
                                csl = slice(1024 * ic + 512 * half,
                                            1024 * ic + 512 * half + 512)
                                nc.tensor.matmul(
                                    bc[64 * hi:64 * hi + 64,
                                       512 * half:512 * half + 512],
                                    ones_sb[32 * h:32 * h + 1, :],
                                    sums_sb[32 * h:32 * h + 1, csl],
                                    start=True,
                                    stop=True,
                                    tile_position=(32 * h, 64 * hi),
                                )
                        rb = qkvpool.tile([128, 1024], F32, tag="rb", bufs=2)
                        nc.vector.reciprocal_approx_fast(rb[:], bc[:])
                        o_sl = oT_sb[:, pair, isl]
                        nc.vector.tensor_tensor(o_sl, o_sl, rb[:], MULT)

            def proj_phase():
                # two t-blocks share one SBUF tile so each y DMA moves 1 MiB
                for tb2 in range(8):
                    y2 = ypool.tile([128, 2, C], F32, tag="yt", name="y2")
                    for sub in range(2):
                        tb = 2 * tb2 + sub
                        py = psa.tile([128, 1024], F32, tag="s", name="py")
                        for nk in range(2):
                            for cp in range(2):
                                nc.tensor.matmul(
                                    py[:, nk * 512:(nk + 1) * 512],
                                    oT_sb[:, cp, tb * 128:(tb + 1) * 128],
                                    wo_sb[:, cp, nk * 512:(nk + 1) * 512],
                                    start=(cp == 0),
                                    stop=(cp == 1),
                                )
                        dst = y2[:, sub, :]
                        if sub == 0:
                            nc.scalar.copy(dst, py[:])
                        else:
                            nc.vector.tensor_copy(dst, py[:])
                    nc.gpsimd.dma_start(
                        y[tb2 * 256:(tb2 + 1) * 256, :].rearrange(
                            "(b p) c -> p b c", p=128
                        ),
                        y2[:],
                    )

            # ---- software-pipelined rep loop ----
            # emission order per rep: [xT(r+1) load] att(r) [qkv(r+1)]
            # norm(r) proj(r) -- next rep's QKV fills the PE during this
            # rep's normalize/proj serialization, and xT transfers during
            # attention.
            xt_cur = xpool.tile([128, 8, T], BF16, tag="xT", bufs=2)
            load_x(xt_cur)
            cur = qkv_phase(xt_cur)
            for _rep in range(nrep):
                if _rep + 1 < nrep:
                    xt_nxt = xpool.tile([128, 8, T], BF16, tag="xT", bufs=2)
                    load_x(xt_nxt)
                att_phase(*cur)
                if _rep + 1 < nrep:
                    cur = qkv_phase(xt_nxt)
                norm_phase()
                proj_phase()

    nc.compile()
    return nc


def _get_nc():
    global _compiled
    if _compiled is None:
        _compiled = _build()
    return _compiled


class _Runner:
    """Compiled PJRT executor for the SPMD kernel, reusable across calls."""

    def __init__(self, nc):
        import jax
        import concourse.mybir as mybir
        from concourse import bass2jax
        from jax.experimental.shard_map import shard_map
        from jax.sharding import Mesh, PartitionSpec

        self.jax = jax
        self.nc = nc
        bass2jax.install_neuronx_cc_hook()

        partition_name = (
            nc.partition_id_tensor.name if nc.partition_id_tensor else None
        )
        in_names, out_names, out_avals, zero_outs = [], [], [], []
        for alloc in nc.m.functions[0].allocations:
            if not isinstance(alloc, mybir.MemoryLocationSet):
                continue
            name = alloc.memorylocations[0].name
            if alloc.kind == "ExternalInput":
                if name != partition_name:
                    in_names.append(name)
            elif alloc.kind == "ExternalOutput":
                out_names.append(name)
                shape = tuple(alloc.tensor_shape)
                dtype = mybir.dt.np(alloc.dtype)
                out_avals.append(jax.core.ShapedArray(shape, dtype))
                zero_outs.append(np.zeros(shape, dtype))
        self.in_names = in_names
        self.out_names = out_names
        self.out_avals = out_avals
        self.zero_outs = zero_outs
        all_names = tuple(in_names + out_names)

        if partition_name is not None:
            all_names = all_names + (partition_name,)

        def _body(*args):
            operands = list(args)
            if partition_name is not None:
                operands.append(bass2jax.partition_id_tensor())
            outs = bass2jax._bass_exec_p.bind(
                *operands,
                out_avals=tuple(out_avals),
                in_names=all_names,
                out_names=tuple(out_names),
                lowering_input_output_aliases=(),
                sim_require_finite=True,
                sim_require_nnan=True,
                nc=nc,
            )
            return tuple(outs)

        devices = jax.devices()[:NCORES]
        assert len(devices) == NCORES
        mesh = Mesh(np.asarray(devices), ("core",))
        self._sharding = jax.sharding.NamedSharding(mesh, PartitionSpec("core"))
        n_args = len(in_names) + len(out_names)
        self.fn = jax.jit(
            shard_map(
                _body,
                mesh=mesh,
                in_specs=(PartitionSpec("core"),) * n_args,
                out_specs=(PartitionSpec("core"),) * len(out_names),
                check_rep=False,
            ),
            keep_unused=True,
        )

    def device_args(self, in_maps):
        args = [
            np.concatenate([np.asarray(m[name]) for m in in_maps], axis=0)
            for name in self.in_names
        ]
        args += [
            np.zeros((NCORES * z.shape[0], *z.shape[1:]), z.dtype)
            for z in self.zero_outs
        ]
        return [self.jax.device_put(a, self._sharding) for a in args]

    def run_device(self, dev_args):
        return self.fn(*dev_args)

    def run(self, in_maps):
        out_arrs = self.fn(*self.device_args(in_maps))
        return [
            {
                name: np.asarray(out_arrs[i]).reshape(
                    NCORES, *self.out_avals[i].shape
                )[c]
                for i, name in enumerate(self.out_names)
            }
            for c in range(NCORES)
        ]


_runner = None


def _get_runner():
    global _runner
    if _runner is None:
        _runner = _Runner(_get_nc())
    return _runner


def make_in_maps(x, Wqkv, Wo):
    import ml_dtypes

    bf16 = ml_dtypes.bfloat16
    x = np.asarray(x, dtype=np.float32)
    Wqkv = np.asarray(Wqkv, dtype=np.float32)
    Wo = np.asarray(Wo, dtype=np.float32)
    mask = np.triu(np.ones((128, 128), dtype=np.float32)).astype(bf16)
    in_maps = []
    for c in range(NCORES):
        b, g = c // 4, c % 4
        in_maps.append({
            "xT": np.ascontiguousarray(x[b].T).astype(bf16),
            "wq": np.ascontiguousarray(
                Wqkv[:, g * CO:(g + 1) * CO]).astype(bf16),
            "wk": np.ascontiguousarray(
                Wqkv[:, C + g * CO:C + (g + 1) * CO]).astype(bf16),
            "wv": np.ascontiguousarray(
                Wqkv[:, 2 * C + g * CO:2 * C + (g + 1) * CO]).astype(bf16),
            "wo": np.ascontiguousarray(Wo[g * CO:(g + 1) * CO, :]).astype(bf16),
            "mask": mask,
        })
    return in_maps


def gather_output(results):
    y = np.zeros((B, T, C), dtype=np.float32)
    for c in range(NCORES):
        y[c // 4] += results[c]["y"]
    return y


def kernel(x, Wqkv, Wo):
    runner = _get_runner()
    in_maps = make_in_maps(x, Wqkv, Wo)
    return gather_output(runner.run(in_maps))


# revision 18
# speedup vs baseline: 1.1435x; 1.0800x over previous
"""Causal self-attention (B=2, T=2048, d_model=1024, H=16) on 8 TRN2 NeuronCores.

Sharding: core c handles batch b = c//4 and head group g = c%4 (heads 4g..4g+3).
Each core computes QKV projection for its heads, causal attention, and a partial
output projection y_partial = attn_out @ Wo[g*256:(g+1)*256, :]. The host sums
the 4 partials per batch (the tensor-parallel all-reduce, done on host).

v2 (bf16 pipeline):
  - all PE operands bf16 (host-cast weights/x), f32 PSUM accumulation.
    FWL fast weight load + warm-clock matmuls.
  - softmax normalization batched at end of attention: denominators DVE-copied
    to 4 contiguous rows, one reciprocal_approx_fast, DMA-scatter to aligned
    partitions {0,32,64,96}, then per (pair, ic) a K=1 PE matmul broadcasts
    1/sum across 64 partitions into PSUM and one DVE mult normalizes both
    heads of the pair at once.  (The old per-pair reciprocal + 256KB broadcast
    DMA blocked the DVE FIFO ~14.5us per pair and let the PE clock re-throttle.)
  - xT prefetched one rep ahead (bufs=2), qT/kT double-buffered so next rep's
    QKV overlaps current rep's attention.

Layouts on device (per core):
  xT  [1024, 2048] bf16 = x[b].T          (contraction dim on partitions)
  qT/kT [128, 2, 2048] bf16               (two heads packed per 128 partitions,
                                           head dim 64 on partitions)
  S^T tiles [128 keys, <=512 queries] f32 PSUM; exp on ACT -> et bf16;
  V' = [V | 1] so the PV matmul also produces per-query denominators.
"""
import sys

sys.path.insert(0, "/opt/trn_rl_repo")

import numpy as np

B, T, C = 2, 2048, 1024
NH_TOT = 16
HD = 64
NH = 4          # heads per core
CO = NH * HD    # 256 channels per core
NCORES = 8
SCALE = 1.0 / 32.0  # d_model ** -0.5

_compiled = None


def _build(nrep=1, trace_sim=False):
    import concourse.bass as bass  # noqa: F401
    import concourse.mybir as mybir
    import concourse.tile as tile
    from concourse import bacc

    F32 = mybir.dt.float32
    F32R = mybir.dt.float32r
    BF16 = mybir.dt.bfloat16
    MULT = mybir.AluOpType.mult
    EXP = mybir.ActivationFunctionType.Exp

    nc = bacc.Bacc("TRN2", target_bir_lowering=False)

    xT = nc.declare_dram_parameter("xT", [C, T], BF16, isOutput=False)
    wq = nc.declare_dram_parameter("wq", [C, CO], BF16, isOutput=False)
    wk = nc.declare_dram_parameter("wk", [C, CO], BF16, isOutput=False)
    wv = nc.declare_dram_parameter("wv", [C, CO], BF16, isOutput=False)
    wo = nc.declare_dram_parameter("wo", [CO, C], BF16, isOutput=False)
    mask = nc.declare_dram_parameter("mask", [128, 128], BF16, isOutput=False)
    y = nc.declare_dram_parameter("y", [T, C], F32, isOutput=True)

    xT_t = xT.rearrange("(o p) t -> p o t", p=128)   # [128, 8, 2048]
    wq_t = wq.rearrange("(o p) m -> p o m", p=128)   # [128, 8, 256]
    wk_t = wk.rearrange("(o p) m -> p o m", p=128)
    wv_t = wv.rearrange("(o p) m -> p o m", p=128)
    wo_t = wo.rearrange("(o p) m -> p o m", p=128)   # [128, 2, 1024]

    with tile.TileContext(nc, trace_sim=trace_sim) as tc:
        with (
            nc.allow_low_precision(reason="bf16 matmul pipeline"),
            tc.tile_pool(name="wpool", bufs=1) as wpool,
            tc.tile_pool(name="qkvpool", bufs=1) as qkvpool,
            tc.tile_pool(name="xpool", bufs=2) as xpool,
            tc.tile_pool(name="etpool", bufs=6) as etpool,
            tc.tile_pool(name="ypool", bufs=2) as ypool,
            tc.tile_pool(name="psa", bufs=2, space="PSUM") as psa,
        ):
            wq_sb = wpool.tile([128, 8, CO], BF16, tag="wq")
            wk_sb = wpool.tile([128, 8, CO], BF16, tag="wk")
            wv_sb = wpool.tile([128, 8, CO], BF16, tag="wv")
            wo_sb = wpool.tile([128, 2, C], BF16, tag="wo")
            mask_sb = wpool.tile([128, 128], BF16, tag="mask")
            ones_sb = wpool.tile([128, HD], BF16, tag="ones")
            nc.sync.dma_start(wq_sb[:], wq_t[:])
            nc.sync.dma_start(wk_sb[:], wk_t[:])
            nc.sync.dma_start(wv_sb[:], wv_t[:])
            nc.sync.dma_start(wo_sb[:], wo_t[:])
            nc.sync.dma_start(mask_sb[:], mask[:])
            nc.vector.memset(ones_sb[:], 1.0)

            # V' ones columns (col HD of each head) are set once per slot
            # below, right after the first allocation of each vp buffer.
            oT_sb = qkvpool.tile([128, 2, 4, 512], BF16, tag="oT")
            sums_sb = qkvpool.tile([128, T], BF16, tag="sums")

            def load_x(xt):
                for th in range(2):
                    for kc in range(8):
                        nc.sync.dma_start(
                            xt[:, kc, th * 1024:(th + 1) * 1024],
                            xT_t[:, kc, th * 1024:(th + 1) * 1024],
                        )

            def qkv_phase(xT_sb):
                """QKV projection; returns (qT, kT, vp) tiles for the rep."""
                qT_sb = qkvpool.tile([128, 2, 4, 512], BF16, tag="qT", bufs=2)
                kT_sb = qkvpool.tile([128, 2, 4, 512], BF16, tag="kT", bufs=2)
                vp_sb = qkvpool.tile([128, 16, NH, HD + 1], BF16, tag="vp",
                                     bufs=2)
                nc.vector.memset(vp_sb[:, :, :, HD], 1.0)

                # qT/kT: [c_out pair on partitions, t free]
                for t8 in range(2):
                    for w_sb, dst in ((wq_sb, qT_sb), (wk_sb, kT_sb)):
                        for m in range(2):
                            pq = psa.tile([128, 2, 512], F32, tag="s",
                                          name="pq")
                            for half in range(2):
                                t0c = t8 * 1024 + half * 512
                                for kc in range(8):
                                    nc.tensor.matmul(
                                        pq[:, half, :],
                                        w_sb[:, kc, m * 128:(m + 1) * 128],
                                        xT_sb[:, kc, t0c:t0c + 512],
                                        start=(kc == 0),
                                        stop=(kc == 7),
                                    )
                            nc.vector.tensor_copy(
                                dst[:, m, 2 * t8:2 * t8 + 2, :], pq[:]
                            )

                # V in [t on partitions, head channels] layout
                for tb in range(16):
                    pv = psa.tile([128, 2, 512], F32, tag="s", name="pv")
                    for kc in range(8):
                        nc.tensor.matmul(
                            pv[:, 0, 0:CO],
                            xT_sb[:, kc, tb * 128:(tb + 1) * 128],
                            wv_sb[:, kc, :],
                            start=(kc == 0),
                            stop=(kc == 7),
                        )
                    nc.vector.tensor_copy(
                        vp_sb[:, tb, :, 0:HD],
                        pv[:, 0, 0:CO].rearrange("p (h d) -> p h d", h=NH),
                    )
                return qT_sb, kT_sb, vp_sb

            def att_phase(qT_sb, kT_sb, vp_sb):
                # merged-head S tiles: ps_s [128 keys, 2 heads, 512 queries];
                # one exp instruction covers both heads via a 3D AP. 512-query
                # chunks keep pos at [65, 512] (1 PSUM bank per head).
                for pair in range(NH // 2):
                    heads = (2 * pair, 2 * pair + 1)
                    for icq in range(4):
                        i_base = 512 * icq
                        jb_last = 4 * icq + 3
                        pos = [
                            psa.tile([65, 512], F32, tag=f"o{hi}",
                                     bufs=1, name=f"po{hi}")
                            for hi in range(2)
                        ]

                        def emit_s(jb):
                            i0 = max(i_base, 128 * jb)
                            o0 = i0 - i_base
                            ps_s = psa.tile([128, 2, 512], F32, tag="s",
                                            bufs=2, name="ps_s")
                            for hi in range(2):
                                nc.tensor.matmul(
                                    ps_s[:, hi, o0:512],
                                    kT_sb[64 * hi:64 * hi + 64, pair, jb // 4,
                                          (jb % 4) * 128:(jb % 4) * 128 + 128],
                                    qT_sb[64 * hi:64 * hi + 64, pair, icq,
                                          o0:512],
                                    start=True,
                                    stop=True,
                                )
                            et = etpool.tile([128, 2, 512], BF16, tag="et",
                                             name="et")
                            nc.scalar.activation(
                                et[:, :, o0:512], ps_s[:, :, o0:512], EXP,
                                scale=SCALE,
                            )
                            if 128 * jb >= i_base:
                                for hi in range(2):
                                    nc.vector.tensor_tensor(
                                        et[:, hi, o0:o0 + 128],
                                        et[:, hi, o0:o0 + 128],
                                        mask_sb[:], MULT,
                                    )
                            return et, o0

                        def emit_pv(jb, et, o0):
                            for hi in range(2):
                                nc.tensor.matmul(
                                    pos[hi][:, o0:512],
                                    vp_sb[:, jb, heads[hi], :],
                                    et[:, hi, o0:512],
                                    start=(jb == 0),
                                    stop=(jb == jb_last),
                                )

                        pending = emit_s(0)
                        for jb in range(jb_last + 1):
                            nxt = emit_s(jb + 1) if jb < jb_last else None
                            emit_pv(jb, *pending)
                            if nxt is not None:
                                pending = nxt

                        # stage unnormalized O^T and the denominators
                        for hi, h in enumerate(heads):
                            nc.vector.tensor_copy(
                                oT_sb[64 * hi:64 * hi + 64, pair, icq, :],
                                pos[hi][0:64, :],
                            )
                            nc.vector.tensor_copy(
                                sums_sb[32 * h:32 * h + 1,
                                        i_base:i_base + 512],
                                pos[hi][64:65, :],
                            )

            def norm_phase():
                # broadcast sums across 64 partitions via K=1 matmuls, then
                # full-lane reciprocal on the [128,2,512] PSUM tile.
                for pair in range(NH // 2):
                    for ic in range(2):
                        bc = psa.tile([128, 2, 512], F32, tag="s", name="bc")
                        for hi in range(2):
                            h = 2 * pair + hi
                            for half in range(2):
                                csl = slice(1024 * ic + 512 * half,
                                            1024 * ic + 512 * half + 512)
                                nc.tensor.matmul(
                                    bc[64 * hi:64 * hi + 64, half, :],
                                    ones_sb[32 * h:32 * h + 1, :],
                                    sums_sb[32 * h:32 * h + 1, csl],
                                    start=True,
                                    stop=True,
                                    tile_position=(32 * h, 64 * hi),
                                )
                        rb = qkvpool.tile([128, 2, 512], F32, tag="rb",
                                          bufs=2)
                        nc.vector.reciprocal_approx_fast(rb[:], bc[:])
                        for half in range(2):
                            o_sl = oT_sb[:, pair, 2 * ic + half, :]
                            nc.vector.tensor_tensor(
                                o_sl, o_sl, rb[:, half, :], MULT
                            )

            def proj_phase():
                # two t-blocks share one SBUF tile; y DMAs issue from the
                # gpsimd queue so they never block the xT prefetch.
                for tb2 in range(8):
                    y2 = ypool.tile([128, 2, 2, 512], F32, tag="yt",
                                    name="y2")
                    for sub in range(2):
                        tb = 2 * tb2 + sub
                        py = psa.tile([128, 2, 512], F32, tag="s", name="py")
                        for nk in range(2):
                            for cp in range(2):
                                nc.tensor.matmul(
                                    py[:, nk, :],
                                    oT_sb[:, cp, tb // 4,
                                          (tb % 4) * 128:(tb % 4) * 128 + 128],
                                    wo_sb[:, cp, nk * 512:(nk + 1) * 512],
                                    start=(cp == 0),
                                    stop=(cp == 1),
                                )
                        dst = y2[:, sub]
                        if sub == 0:
                            nc.scalar.copy(dst, py[:])
                        else:
                            nc.vector.tensor_copy(dst, py[:])
                    for sub in range(2):
                        tb = 2 * tb2 + sub
                        nc.gpsimd.dma_start(
                            y[tb * 128:(tb + 1) * 128, :],
                            y2[:, sub],
                        )

            # ---- software-pipelined rep loop ----
            # emission order per rep: [xT(r+1) load] att(r) [qkv(r+1)]
            # norm(r) proj(r) -- next rep's QKV fills the PE during this
            # rep's normalize/proj serialization, and xT transfers during
            # attention.
            xt_cur = xpool.tile([128, 8, T], BF16, tag="xT", bufs=2)
            load_x(xt_cur)
            cur = qkv_phase(xt_cur)
            for _rep in range(nrep):
                if _rep + 1 < nrep:
                    xt_nxt = xpool.tile([128, 8, T], BF16, tag="xT", bufs=2)
                    load_x(xt_nxt)
                att_phase(*cur)
                if _rep + 1 < nrep:
                    cur = qkv_phase(xt_nxt)
                norm_phase()
                proj_phase()

    nc.compile()
    return nc


def _get_nc():
    global _compiled
    if _compiled is None:
        _compiled = _build()
    return _compiled


class _Runner:
    """Compiled PJRT executor for the SPMD kernel, reusable across calls."""

    def __init__(self, nc):
        import jax
        import concourse.mybir as mybir
        from concourse import bass2jax
        from jax.experimental.shard_map import shard_map
        from jax.sharding import Mesh, PartitionSpec

        self.jax = jax
        self.nc = nc
        bass2jax.install_neuronx_cc_hook()

        partition_name = (
            nc.partition_id_tensor.name if nc.partition_id_tensor else None
        )
        in_names, out_names, out_avals, zero_outs = [], [], [], []
        for alloc in nc.m.functions[0].allocations:
            if not isinstance(alloc, mybir.MemoryLocationSet):
                continue
            name = alloc.memorylocations[0].name
            if alloc.kind == "ExternalInput":
                if name != partition_name:
                    in_names.append(name)
            elif alloc.kind == "ExternalOutput":
                out_names.append(name)
                shape = tuple(alloc.tensor_shape)
                dtype = mybir.dt.np(alloc.dtype)
                out_avals.append(jax.core.ShapedArray(shape, dtype))
                zero_outs.append(np.zeros(shape, dtype))
        self.in_names = in_names
        self.out_names = out_names
        self.out_avals = out_avals
        self.zero_outs = zero_outs
        all_names = tuple(in_names + out_names)

        if partition_name is not None:
            all_names = all_names + (partition_name,)

        def _body(*args):
            operands = list(args)
            if partition_name is not None:
                operands.append(bass2jax.partition_id_tensor())
            outs = bass2jax._bass_exec_p.bind(
                *operands,
                out_avals=tuple(out_avals),
                in_names=all_names,
                out_names=tuple(out_names),
                lowering_input_output_aliases=(),
                sim_require_finite=True,
                sim_require_nnan=True,
                nc=nc,
            )
            return tuple(outs)

        devices = jax.devices()[:NCORES]
        assert len(devices) == NCORES
        mesh = Mesh(np.asarray(devices), ("core",))
        self._sharding = jax.sharding.NamedSharding(mesh, PartitionSpec("core"))
        n_args = len(in_names) + len(out_names)
        self.fn = jax.jit(
            shard_map(
                _body,
                mesh=mesh,
                in_specs=(PartitionSpec("core"),) * n_args,
                out_specs=(PartitionSpec("core"),) * len(out_names),
                check_rep=False,
            ),
            keep_unused=True,
        )

    def device_args(self, in_maps):
        args = [
            np.concatenate([np.asarray(m[name]) for m in in_maps], axis=0)
            for name in self.in_names
        ]
        args += [
            np.zeros((NCORES * z.shape[0], *z.shape[1:]), z.dtype)
            for z in self.zero_outs
        ]
        return [self.jax.device_put(a, self._sharding) for a in args]

    def run_device(self, dev_args):
        return self.fn(*dev_args)

    def run(self, in_maps):
        out_arrs = self.fn(*self.device_args(in_maps))
        return [
            {
                name: np.asarray(out_arrs[i]).reshape(
                    NCORES, *self.out_avals[i].shape
                )[c]
                for i, name in enumerate(self.out_names)
            }
            for c in range(NCORES)
        ]


_runner = None


def _get_runner():
    global _runner
    if _runner is None:
        _runner = _Runner(_get_nc())
    return _runner


def make_in_maps(x, Wqkv, Wo):
    import ml_dtypes

    bf16 = ml_dtypes.bfloat16
    x = np.asarray(x, dtype=np.float32)
    Wqkv = np.asarray(Wqkv, dtype=np.float32)
    Wo = np.asarray(Wo, dtype=np.float32)
    mask = np.triu(np.ones((128, 128), dtype=np.float32)).astype(bf16)
    in_maps = []
    for c in range(NCORES):
        b, g = c // 4, c % 4
        in_maps.append({
            "xT": np.ascontiguousarray(x[b].T).astype(bf16),
            "wq": np.ascontiguousarray(
                Wqkv[:, g * CO:(g + 1) * CO]).astype(bf16),
            "wk": np.ascontiguousarray(
                Wqkv[:, C + g * CO:C + (g + 1) * CO]).astype(bf16),
            "wv": np.ascontiguousarray(
                Wqkv[:, 2 * C + g * CO:2 * C + (g + 1) * CO]).astype(bf16),
            "wo": np.ascontiguousarray(Wo[g * CO:(g + 1) * CO, :]).astype(bf16),
            "mask": mask,
        })
    return in_maps


def gather_output(results):
    y = np.zeros((B, T, C), dtype=np.float32)
    for c in range(NCORES):
        y[c // 4] += results[c]["y"]
    return y


def kernel(x, Wqkv, Wo):
    runner = _get_runner()
    in_maps = make_in_maps(x, Wqkv, Wo)
    return gather_output(runner.run(in_maps))


# revision 19
# speedup vs baseline: 1.6779x; 1.4673x over previous
"""Causal self-attention (B=2, T=2048, d_model=1024, H=16) on 8 TRN2 NeuronCores.

Sharding: core c handles batch b = c//4 and head group g = c%4 (heads 4g..4g+3).
Each core computes QKV projection for its heads, causal attention, and a partial
output projection y_partial = attn_out @ Wo[g*256:(g+1)*256, :]. The host sums
the 4 partials per batch (the tensor-parallel all-reduce, done on host).

v2 (bf16 pipeline):
  - all PE operands bf16 (host-cast weights/x), f32 PSUM accumulation.
    FWL fast weight load + warm-clock matmuls.
  - softmax normalization batched at end of attention: denominators DVE-copied
    to 4 contiguous rows, one reciprocal_approx_fast, DMA-scatter to aligned
    partitions {0,32,64,96}, then per (pair, ic) a K=1 PE matmul broadcasts
    1/sum across 64 partitions into PSUM and one DVE mult normalizes both
    heads of the pair at once.  (The old per-pair reciprocal + 256KB broadcast
    DMA blocked the DVE FIFO ~14.5us per pair and let the PE clock re-throttle.)
  - xT prefetched one rep ahead (bufs=2), qT/kT double-buffered so next rep's
    QKV overlaps current rep's attention.

Layouts on device (per core):
  xT  [1024, 2048] bf16 = x[b].T          (contraction dim on partitions)
  qT/kT [128, 2, 2048] bf16               (two heads packed per 128 partitions,
                                           head dim 64 on partitions)
  S^T tiles [128 keys, <=512 queries] f32 PSUM; exp on ACT -> et bf16;
  V' = [V | 1] so the PV matmul also produces per-query denominators.
"""
import sys

sys.path.insert(0, "/opt/trn_rl_repo")

import numpy as np

B, T, C = 2, 2048, 1024
NH_TOT = 16
HD = 64
NH = 4          # heads per core
CO = NH * HD    # 256 channels per core
NCORES = 8
SCALE = 1.0 / 32.0  # d_model ** -0.5

_compiled = None


def _build(nrep=1, trace_sim=False):
    import concourse.bass as bass  # noqa: F401
    import concourse.mybir as mybir
    import concourse.tile as tile
    from concourse import bacc

    F32 = mybir.dt.float32
    F32R = mybir.dt.float32r
    BF16 = mybir.dt.bfloat16
    MULT = mybir.AluOpType.mult
    EXP = mybir.ActivationFunctionType.Exp

    nc = bacc.Bacc("TRN2", target_bir_lowering=False)

    xT = nc.declare_dram_parameter("xT", [C, T], BF16, isOutput=False)
    wq = nc.declare_dram_parameter("wq", [C, CO], BF16, isOutput=False)
    wk = nc.declare_dram_parameter("wk", [C, CO], BF16, isOutput=False)
    wv = nc.declare_dram_parameter("wv", [C, CO], BF16, isOutput=False)
    wo = nc.declare_dram_parameter("wo", [CO, C], BF16, isOutput=False)
    mask = nc.declare_dram_parameter("mask", [128, 128], BF16, isOutput=False)
    y = nc.declare_dram_parameter("y", [T, C], F32, isOutput=True)

    xT_t = xT.rearrange("(o p) t -> p o t", p=128)   # [128, 8, 2048]
    wq_t = wq.rearrange("(o p) m -> p o m", p=128)   # [128, 8, 256]
    wk_t = wk.rearrange("(o p) m -> p o m", p=128)
    wv_t = wv.rearrange("(o p) m -> p o m", p=128)
    wo_t = wo.rearrange("(o p) m -> p o m", p=128)   # [128, 2, 1024]

    with tile.TileContext(nc, trace_sim=trace_sim) as tc:
        with (
            nc.allow_low_precision(reason="bf16 matmul pipeline"),
            tc.tile_pool(name="wpool", bufs=1) as wpool,
            tc.tile_pool(name="qkvpool", bufs=1) as qkvpool,
            tc.tile_pool(name="xpool", bufs=2) as xpool,
            tc.tile_pool(name="etpool", bufs=6) as etpool,
            tc.tile_pool(name="ypool", bufs=2) as ypool,
            tc.tile_pool(name="psa", bufs=2, space="PSUM") as psa,
        ):
            wq_sb = wpool.tile([128, 8, CO], BF16, tag="wq")
            wk_sb = wpool.tile([128, 8, CO], BF16, tag="wk")
            wv_sb = wpool.tile([128, 8, CO], BF16, tag="wv")
            wo_sb = wpool.tile([128, 2, C], BF16, tag="wo")
            mask_sb = wpool.tile([128, 128], BF16, tag="mask")
            ones_sb = wpool.tile([128, HD], BF16, tag="ones")
            nc.sync.dma_start(wq_sb[:], wq_t[:])
            nc.sync.dma_start(wk_sb[:], wk_t[:])
            nc.sync.dma_start(wv_sb[:], wv_t[:])
            nc.sync.dma_start(wo_sb[:], wo_t[:])
            nc.sync.dma_start(mask_sb[:], mask[:])
            nc.vector.memset(ones_sb[:], 1.0)

            # V' ones columns (col HD of each head) are set once per slot
            # below, right after the first allocation of each vp buffer.
            # oT/sums are per-rep tiles (bufs=2) so rep r-1's normalize/proj
            # can run as PE filler inside rep r's attention.

            def load_x(xt):
                for th in range(2):
                    for kc in range(8):
                        nc.sync.dma_start(
                            xt[:, kc, th * 1024:(th + 1) * 1024],
                            xT_t[:, kc, th * 1024:(th + 1) * 1024],
                        )

            def qkv_units(xT_sb):
                """QKV projection as filler units; returns ((qT,kT,vp), units)."""
                qT_sb = qkvpool.tile([128, 2, 4, 512], BF16, tag="qT", bufs=2)
                kT_sb = qkvpool.tile([128, 2, 4, 512], BF16, tag="kT", bufs=2)
                vp_sb = qkvpool.tile([128, 16, NH, HD + 1], BF16, tag="vp",
                                     bufs=2)
                units = [lambda: nc.vector.memset(vp_sb[:, :, :, HD], 1.0)]

                def qk_unit(w_sb, dst, m, blk):
                    def u():
                        pq = psa.tile([128, 512], F32, tag="g", bufs=2,
                                      name="pq")
                        for kc in range(8):
                            nc.tensor.matmul(
                                pq[:],
                                w_sb[:, kc, m * 128:(m + 1) * 128],
                                xT_sb[:, kc, blk * 512:(blk + 1) * 512],
                                start=(kc == 0),
                                stop=(kc == 7),
                            )
                        nc.vector.tensor_copy(dst[:, m, blk, :], pq[:])
                    return u

                def v_unit(tb):
                    def u():
                        pv = psa.tile([128, 512], F32, tag="g", bufs=2,
                                      name="pv")
                        for kc in range(8):
                            nc.tensor.matmul(
                                pv[:, 0:CO],
                                xT_sb[:, kc, tb * 128:(tb + 1) * 128],
                                wv_sb[:, kc, :],
                                start=(kc == 0),
                                stop=(kc == 7),
                            )
                        nc.vector.tensor_copy(
                            vp_sb[:, tb, :, 0:HD],
                            pv[:, 0:CO].rearrange("p (h d) -> p h d", h=NH),
                        )
                    return u

                for blk in range(4):
                    for w_sb, dst in ((wq_sb, qT_sb), (wk_sb, kT_sb)):
                        for m in range(2):
                            units.append(qk_unit(w_sb, dst, m, blk))
                for tb in range(16):
                    units.append(v_unit(tb))
                return (qT_sb, kT_sb, vp_sb), units

            def norm_units(oT_p, sums_p):
                # broadcast sums via K=1 matmul, full-lane approx reciprocal,
                # one multiply normalizes both heads of a pair per 512-block.
                units = []

                def n_unit(pair, blk):
                    def u():
                        bc = psa.tile([128, 512], F32, tag="g", bufs=2,
                                      name="bc")
                        csl = slice(512 * blk, 512 * blk + 512)
                        for hi in range(2):
                            h = 2 * pair + hi
                            nc.tensor.matmul(
                                bc[64 * hi:64 * hi + 64, :],
                                ones_sb[32 * h:32 * h + 1, :],
                                sums_p[32 * h:32 * h + 1, csl],
                                start=True,
                                stop=True,
                                tile_position=(32 * h, 64 * hi),
                            )
                        rb = qkvpool.tile([128, 512], F32, tag="rb", bufs=2)
                        nc.vector.reciprocal_approx_fast(rb[:], bc[:])
                        o_sl = oT_p[:, pair, blk, :]
                        nc.vector.tensor_tensor(o_sl, o_sl, rb[:], MULT)
                    return u

                for pair in range(NH // 2):
                    for blk in range(4):
                        units.append(n_unit(pair, blk))
                return units

            def proj_units(oT_p):
                units = []
                state = {}

                def p_unit(tb2, sub, nk):
                    def u():
                        if sub == 0 and nk == 0:
                            state['y2'] = ypool.tile([128, 2, 2, 512], F32,
                                                     tag="yt", name="y2")
                        y2 = state['y2']
                        tb = 2 * tb2 + sub
                        py = psa.tile([128, 512], F32, tag="g", bufs=2,
                                      name="py")
                        for cp in range(2):
                            nc.tensor.matmul(
                                py[:],
                                oT_p[:, cp, tb // 4,
                                     (tb % 4) * 128:(tb % 4) * 128 + 128],
                                wo_sb[:, cp, nk * 512:(nk + 1) * 512],
                                start=(cp == 0),
                                stop=(cp == 1),
                            )
                        dst = y2[:, sub, nk, :]
                        if nk == 0:
                            nc.scalar.copy(dst, py[:])
                        else:
                            nc.vector.tensor_copy(dst, py[:])
                        if sub == 1 and nk == 1:
                            for s2 in range(2):
                                tb3 = 2 * tb2 + s2
                                nc.gpsimd.dma_start(
                                    y[tb3 * 128:(tb3 + 1) * 128, :],
                                    y2[:, s2],
                                )
                    return u

                for tb2 in range(8):
                    for sub in range(2):
                        for nk in range(2):
                            units.append(p_unit(tb2, sub, nk))
                return units

            def att_phase(qT_sb, kT_sb, vp_sb, fillers):
                # merged-head S tiles: ps_s [128 keys, 2 heads, 512 queries];
                # one exp instruction covers both heads via a 3D AP. 512-query
                # chunks keep pos at [65, 512] (1 PSUM bank per head). One
                # filler unit (prev-rep norm/proj, next-rep QKV) is emitted
                # per jb iteration to keep the PE busy through ACT stalls.
                oT_l = qkvpool.tile([128, 2, 4, 512], BF16, tag="oT", bufs=2)
                sums_l = qkvpool.tile([128, T], BF16, tag="sums", bufs=2)
                fit = iter(fillers)

                def fill():
                    u = next(fit, None)
                    if u is not None:
                        u()

                for pair in range(NH // 2):
                    heads = (2 * pair, 2 * pair + 1)
                    for icq in range(4):
                        i_base = 512 * icq
                        jb_last = 4 * icq + 3
                        pos = [
                            psa.tile([65, 512], F32, tag=f"o{hi}",
                                     bufs=1, name=f"po{hi}")
                            for hi in range(2)
                        ]

                        def emit_s(jb):
                            i0 = max(i_base, 128 * jb)
                            o0 = i0 - i_base
                            ps_s = psa.tile([128, 2, 512], F32, tag="s",
                                            bufs=2, name="ps_s")
                            for hi in range(2):
                                nc.tensor.matmul(
                                    ps_s[:, hi, o0:512],
                                    kT_sb[64 * hi:64 * hi + 64, pair, jb // 4,
                                          (jb % 4) * 128:(jb % 4) * 128 + 128],
                                    qT_sb[64 * hi:64 * hi + 64, pair, icq,
                                          o0:512],
                                    start=True,
                                    stop=True,
                                )
                            et = etpool.tile([128, 2, 512], BF16, tag="et",
                                             name="et")
                            nc.scalar.activation(
                                et[:, :, o0:512], ps_s[:, :, o0:512], EXP,
                                scale=SCALE,
                            )
                            if 128 * jb >= i_base:
                                for hi in range(2):
                                    nc.vector.tensor_tensor(
                                        et[:, hi, o0:o0 + 128],
                                        et[:, hi, o0:o0 + 128],
                                        mask_sb[:], MULT,
                                    )
                            return et, o0

                        def emit_pv(jb, et, o0):
                            for hi in range(2):
                                nc.tensor.matmul(
                                    pos[hi][:, o0:512],
                                    vp_sb[:, jb, heads[hi], :],
                                    et[:, hi, o0:512],
                                    start=(jb == 0),
                                    stop=(jb == jb_last),
                                )

                        pending = emit_s(0)
                        for jb in range(jb_last + 1):
                            nxt = emit_s(jb + 1) if jb < jb_last else None
                            emit_pv(jb, *pending)
                            fill()
                            if nxt is not None:
                                pending = nxt

                        # stage unnormalized O^T and the denominators
                        for hi, h in enumerate(heads):
                            nc.vector.tensor_copy(
                                oT_l[64 * hi:64 * hi + 64, pair, icq, :],
                                pos[hi][0:64, :],
                            )
                            nc.vector.tensor_copy(
                                sums_l[32 * h:32 * h + 1,
                                       i_base:i_base + 512],
                                pos[hi][64:65, :],
                            )
                        fill()

                for u in fit:
                    u()
                return oT_l, sums_l

            # ---- software-pipelined rep loop ----
            # rep r's attention interleaves: rep r-1's normalize + output
            # projection and rep r+1's QKV, one unit per jb iteration.
            xt_cur = xpool.tile([128, 8, T], BF16, tag="xT", bufs=2)
            load_x(xt_cur)
            cur, units0 = qkv_units(xt_cur)
            for u in units0:
                u()
            prev = None
            for _rep in range(nrep):
                if _rep + 1 < nrep:
                    xt_nxt = xpool.tile([128, 8, T], BF16, tag="xT", bufs=2)
                    load_x(xt_nxt)
                fillers = []
                if prev is not None:
                    fillers += norm_units(*prev)
                    fillers += proj_units(prev[0])
                nxt_tiles = None
                if _rep + 1 < nrep:
                    nxt_tiles, qunits = qkv_units(xt_nxt)
                    fillers += qunits
                prev = att_phase(*cur, fillers)
                if nxt_tiles is not None:
                    cur = nxt_tiles
            for u in norm_units(*prev) + proj_units(prev[0]):
                u()

    nc.compile()
    return nc


def _get_nc():
    global _compiled
    if _compiled is None:
        _compiled = _build()
    return _compiled


class _Runner:
    """Compiled PJRT executor for the SPMD kernel, reusable across calls."""

    def __init__(self, nc):
        import jax
        import concourse.mybir as mybir
        from concourse import bass2jax
        from jax.experimental.shard_map import shard_map
        from jax.sharding import Mesh, PartitionSpec

        self.jax = jax
        self.nc = nc
        bass2jax.install_neuronx_cc_hook()

        partition_name = (
            nc.partition_id_tensor.name if nc.partition_id_tensor else None
        )
        in_names, out_names, out_avals, zero_outs = [], [], [], []
        for alloc in nc.m.functions[0].allocations:
            if not isinstance(alloc, mybir.MemoryLocationSet):
                continue
            name = alloc.memorylocations[0].name
            if alloc.kind == "ExternalInput":
                if name != partition_name:
                    in_names.append(name)
            elif alloc.kind == "ExternalOutput":
                out_names.append(name)
                shape = tuple(alloc.tensor_shape)
                dtype = mybir.dt.np(alloc.dtype)
                out_avals.append(jax.core.ShapedArray(shape, dtype))
                zero_outs.append(np.zeros(shape, dtype))
        self.in_names = in_names
        self.out_names = out_names
        self.out_avals = out_avals
        self.zero_outs = zero_outs
        all_names = tuple(in_names + out_names)

        if partition_name is not None:
            all_names = all_names + (partition_name,)

        def _body(*args):
            operands = list(args)
            if partition_name is not None:
                operands.append(bass2jax.partition_id_tensor())
            outs = bass2jax._bass_exec_p.bind(
                *operands,
                out_avals=tuple(out_avals),
                in_names=all_names,
                out_names=tuple(out_names),
                lowering_input_output_aliases=(),
                sim_require_finite=True,
                sim_require_nnan=True,
                nc=nc,
            )
            return tuple(outs)

        devices = jax.devices()[:NCORES]
        assert len(devices) == NCORES
        mesh = Mesh(np.asarray(devices), ("core",))
        self._sharding = jax.sharding.NamedSharding(mesh, PartitionSpec("core"))
        n_args = len(in_names) + len(out_names)
        self.fn = jax.jit(
            shard_map(
                _body,
                mesh=mesh,
                in_specs=(PartitionSpec("core"),) * n_args,
                out_specs=(PartitionSpec("core"),) * len(out_names),
                check_rep=False,
            ),
            keep_unused=True,
        )

    def device_args(self, in_maps):
        args = [
            np.concatenate([np.asarray(m[name]) for m in in_maps], axis=0)
            for name in self.in_names
        ]
        args += [
            np.zeros((NCORES * z.shape[0], *z.shape[1:]), z.dtype)
            for z in self.zero_outs
        ]
        return [self.jax.device_put(a, self._sharding) for a in args]

    def run_device(self, dev_args):
        return self.fn(*dev_args)

    def run(self, in_maps):
        out_arrs = self.fn(*self.device_args(in_maps))
        return [
            {
                name: np.asarray(out_arrs[i]).reshape(
                    NCORES, *self.out_avals[i].shape
                )[c]
                for i, name in enumerate(self.out_names)
            }
            for c in range(NCORES)
        ]


_runner = None


def _get_runner():
    global _runner
    if _runner is None:
        _runner = _Runner(_get_nc())
    return _runner


def make_in_maps(x, Wqkv, Wo):
    import ml_dtypes

    bf16 = ml_dtypes.bfloat16
    x = np.asarray(x, dtype=np.float32)
    Wqkv = np.asarray(Wqkv, dtype=np.float32)
    Wo = np.asarray(Wo, dtype=np.float32)
    mask = np.triu(np.ones((128, 128), dtype=np.float32)).astype(bf16)
    in_maps = []
    for c in range(NCORES):
        b, g = c // 4, c % 4
        in_maps.append({
            "xT": np.ascontiguousarray(x[b].T).astype(bf16),
            "wq": np.ascontiguousarray(
                Wqkv[:, g * CO:(g + 1) * CO]).astype(bf16),
            "wk": np.ascontiguousarray(
                Wqkv[:, C + g * CO:C + (g + 1) * CO]).astype(bf16),
            "wv": np.ascontiguousarray(
                Wqkv[:, 2 * C + g * CO:2 * C + (g + 1) * CO]).astype(bf16),
            "wo": np.ascontiguousarray(Wo[g * CO:(g + 1) * CO, :]).astype(bf16),
            "mask": mask,
        })
    return in_maps


def gather_output(results):
    y = np.zeros((B, T, C), dtype=np.float32)
    for c in range(NCORES):
        y[c // 4] += results[c]["y"]
    return y


def kernel(x, Wqkv, Wo):
    runner = _get_runner()
    in_maps = make_in_maps(x, Wqkv, Wo)
    return gather_output(runner.run(in_maps))


# revision 20
# speedup vs baseline: 1.7443x; 1.0396x over previous
"""Causal self-attention (B=2, T=2048, d_model=1024, H=16) on 8 TRN2 NeuronCores.

Sharding: core c handles batch b = c//4 and head group g = c%4 (heads 4g..4g+3).
Each core computes QKV projection for its heads, causal attention, and a partial
output projection y_partial = attn_out @ Wo[g*256:(g+1)*256, :]. The host sums
the 4 partials per batch (the tensor-parallel all-reduce, done on host).

v2 (bf16 pipeline):
  - all PE operands bf16 (host-cast weights/x), f32 PSUM accumulation.
    FWL fast weight load + warm-clock matmuls.
  - softmax normalization batched at end of attention: denominators DVE-copied
    to 4 contiguous rows, one reciprocal_approx_fast, DMA-scatter to aligned
    partitions {0,32,64,96}, then per (pair, ic) a K=1 PE matmul broadcasts
    1/sum across 64 partitions into PSUM and one DVE mult normalizes both
    heads of the pair at once.  (The old per-pair reciprocal + 256KB broadcast
    DMA blocked the DVE FIFO ~14.5us per pair and let the PE clock re-throttle.)
  - xT prefetched one rep ahead (bufs=2), qT/kT double-buffered so next rep's
    QKV overlaps current rep's attention.

Layouts on device (per core):
  xT  [1024, 2048] bf16 = x[b].T          (contraction dim on partitions)
  qT/kT [128, 2, 2048] bf16               (two heads packed per 128 partitions,
                                           head dim 64 on partitions)
  S^T tiles [128 keys, <=512 queries] f32 PSUM; exp on ACT -> et bf16;
  V' = [V | 1] so the PV matmul also produces per-query denominators.
"""
import sys

sys.path.insert(0, "/opt/trn_rl_repo")

import numpy as np

B, T, C = 2, 2048, 1024
NH_TOT = 16
HD = 64
NH = 4          # heads per core
CO = NH * HD    # 256 channels per core
NCORES = 8
SCALE = 1.0 / 32.0  # d_model ** -0.5

_compiled = None


def _build(nrep=1, trace_sim=False):
    import concourse.bass as bass  # noqa: F401
    import concourse.mybir as mybir
    import concourse.tile as tile
    from concourse import bacc

    F32 = mybir.dt.float32
    F32R = mybir.dt.float32r
    BF16 = mybir.dt.bfloat16
    MULT = mybir.AluOpType.mult
    EXP = mybir.ActivationFunctionType.Exp

    nc = bacc.Bacc("TRN2", target_bir_lowering=False)

    xT = nc.declare_dram_parameter("xT", [C, T], BF16, isOutput=False)
    wq = nc.declare_dram_parameter("wq", [C, CO], BF16, isOutput=False)
    wk = nc.declare_dram_parameter("wk", [C, CO], BF16, isOutput=False)
    wv = nc.declare_dram_parameter("wv", [C, CO], BF16, isOutput=False)
    wo = nc.declare_dram_parameter("wo", [CO, C], BF16, isOutput=False)
    mask = nc.declare_dram_parameter("mask", [128, 128], BF16, isOutput=False)
    y = nc.declare_dram_parameter("y", [T, C], F32, isOutput=True)

    xT_t = xT.rearrange("(o p) t -> p o t", p=128)   # [128, 8, 2048]
    wq_t = wq.rearrange("(o p) m -> p o m", p=128)   # [128, 8, 256]
    wk_t = wk.rearrange("(o p) m -> p o m", p=128)
    wv_t = wv.rearrange("(o p) m -> p o m", p=128)
    wo_t = wo.rearrange("(o p) m -> p o m", p=128)   # [128, 2, 1024]

    with tile.TileContext(nc, trace_sim=trace_sim) as tc:
        with (
            nc.allow_low_precision(reason="bf16 matmul pipeline"),
            tc.tile_pool(name="wpool", bufs=1) as wpool,
            tc.tile_pool(name="qkvpool", bufs=1) as qkvpool,
            tc.tile_pool(name="xpool", bufs=2) as xpool,
            tc.tile_pool(name="etpool", bufs=6) as etpool,
            tc.tile_pool(name="ypool", bufs=2) as ypool,
            tc.tile_pool(name="psa", bufs=2, space="PSUM") as psa,
        ):
            wq_sb = wpool.tile([128, 8, CO], BF16, tag="wq")
            wk_sb = wpool.tile([128, 8, CO], BF16, tag="wk")
            wv_sb = wpool.tile([128, 8, CO], BF16, tag="wv")
            wo_sb = wpool.tile([128, 2, C], BF16, tag="wo")
            mask_sb = wpool.tile([128, 128], BF16, tag="mask")
            ones_sb = wpool.tile([128, HD], BF16, tag="ones")
            nc.sync.dma_start(wq_sb[:], wq_t[:])
            nc.sync.dma_start(wk_sb[:], wk_t[:])
            nc.sync.dma_start(wv_sb[:], wv_t[:])
            nc.sync.dma_start(wo_sb[:], wo_t[:])
            nc.sync.dma_start(mask_sb[:], mask[:])
            nc.vector.memset(ones_sb[:], 1.0)

            # V' ones columns (col HD of each head) are set once per slot
            # below, right after the first allocation of each vp buffer.
            # oT/sums are per-rep tiles (bufs=2) so rep r-1's normalize/proj
            # can run as PE filler inside rep r's attention.

            def load_x(xt):
                for th in range(2):
                    for kc in range(8):
                        nc.sync.dma_start(
                            xt[:, kc, th * 1024:(th + 1) * 1024],
                            xT_t[:, kc, th * 1024:(th + 1) * 1024],
                        )

            def qkv_units(xT_sb):
                """QKV projection as filler units; returns ((qT,kT,vp), units)."""
                qT_sb = qkvpool.tile([128, 2, 4, 512], BF16, tag="qT", bufs=2)
                kT_sb = qkvpool.tile([128, 2, 4, 512], BF16, tag="kT", bufs=2)
                vp_sb = qkvpool.tile([128, 16, NH, HD + 1], BF16, tag="vp",
                                     bufs=2)
                units = [lambda: nc.vector.memset(vp_sb[:, :, :, HD], 1.0)]

                def qk_unit(w_sb, dst, m, blk):
                    def u():
                        pq = psa.tile([128, 512], F32, tag="g", bufs=2,
                                      name="pq")
                        for kc in range(8):
                            nc.tensor.matmul(
                                pq[:],
                                w_sb[:, kc, m * 128:(m + 1) * 128],
                                xT_sb[:, kc, blk * 512:(blk + 1) * 512],
                                start=(kc == 0),
                                stop=(kc == 7),
                            )
                        nc.vector.tensor_copy(dst[:, m, blk, :], pq[:])
                    return u

                def v_unit(tb):
                    def u():
                        pv = psa.tile([128, 512], F32, tag="g", bufs=2,
                                      name="pv")
                        for kc in range(8):
                            nc.tensor.matmul(
                                pv[:, 0:CO],
                                xT_sb[:, kc, tb * 128:(tb + 1) * 128],
                                wv_sb[:, kc, :],
                                start=(kc == 0),
                                stop=(kc == 7),
                            )
                        nc.vector.tensor_copy(
                            vp_sb[:, tb, :, 0:HD],
                            pv[:, 0:CO].rearrange("p (h d) -> p h d", h=NH),
                        )
                    return u

                for blk in range(4):
                    for w_sb, dst in ((wq_sb, qT_sb), (wk_sb, kT_sb)):
                        for m in range(2):
                            units.append(qk_unit(w_sb, dst, m, blk))
                for tb in range(16):
                    units.append(v_unit(tb))
                return (qT_sb, kT_sb, vp_sb), units

            def norm_units(oT_p, sums_p):
                # broadcast sums via K=1 matmul, full-lane approx reciprocal,
                # one multiply normalizes both heads of a pair per 512-block.
                units = []

                def n_unit(pair, blk):
                    def u():
                        bc = psa.tile([128, 512], F32, tag="g", bufs=2,
                                      name="bc")
                        csl = slice(512 * blk, 512 * blk + 512)
                        for hi in range(2):
                            h = 2 * pair + hi
                            nc.tensor.matmul(
                                bc[64 * hi:64 * hi + 64, :],
                                ones_sb[32 * h:32 * h + 1, :],
                                sums_p[32 * h:32 * h + 1, csl],
                                start=True,
                                stop=True,
                                tile_position=(32 * h, 64 * hi),
                            )
                        rb = qkvpool.tile([128, 512], F32, tag="rb", bufs=2)
                        nc.vector.reciprocal_approx_fast(rb[:], bc[:])
                        o_sl = oT_p[:, pair, blk, :]
                        nc.vector.tensor_tensor(o_sl, o_sl, rb[:], MULT)
                    return u

                for pair in range(NH // 2):
                    for blk in range(4):
                        units.append(n_unit(pair, blk))
                return units

            def proj_units(oT_p):
                units = []
                state = {}

                def p_unit(tb2, sub, nk):
                    def u():
                        if sub == 0 and nk == 0:
                            state['y2'] = ypool.tile([128, 2, 2, 512], F32,
                                                     tag="yt", name="y2")
                        y2 = state['y2']
                        tb = 2 * tb2 + sub
                        py = psa.tile([128, 512], F32, tag="g", bufs=2,
                                      name="py")
                        for cp in range(2):
                            nc.tensor.matmul(
                                py[:],
                                oT_p[:, cp, tb // 4,
                                     (tb % 4) * 128:(tb % 4) * 128 + 128],
                                wo_sb[:, cp, nk * 512:(nk + 1) * 512],
                                start=(cp == 0),
                                stop=(cp == 1),
                            )
                        dst = y2[:, sub, nk, :]
                        nc.vector.tensor_copy(dst, py[:])
                        if sub == 1 and nk == 1:
                            for s2 in range(2):
                                tb3 = 2 * tb2 + s2
                                nc.gpsimd.dma_start(
                                    y[tb3 * 128:(tb3 + 1) * 128, :],
                                    y2[:, s2],
                                )
                    return u

                for tb2 in range(8):
                    for sub in range(2):
                        for nk in range(2):
                            units.append(p_unit(tb2, sub, nk))
                return units

            def att_phase(qT_sb, kT_sb, vp_sb, fillers):
                # merged-head S tiles: ps_s [128 keys, 2 heads, 512 queries];
                # one exp instruction covers both heads via a 3D AP. 512-query
                # chunks keep pos at [65, 512] (1 PSUM bank per head). One
                # filler unit (prev-rep norm/proj, next-rep QKV) is emitted
                # per jb iteration to keep the PE busy through ACT stalls.
                oT_l = qkvpool.tile([128, 2, 4, 512], BF16, tag="oT", bufs=2)
                sums_l = qkvpool.tile([128, T], BF16, tag="sums", bufs=2)
                fit = iter(fillers)

                def fill():
                    u = next(fit, None)
                    if u is not None:
                        u()

                for pair in range(NH // 2):
                    heads = (2 * pair, 2 * pair + 1)
                    for icq in range(4):
                        i_base = 512 * icq
                        jb_last = 4 * icq + 3
                        pos = [
                            psa.tile([65, 512], F32, tag=f"o{hi}",
                                     bufs=1, name=f"po{hi}")
                            for hi in range(2)
                        ]

                        def emit_s(jb):
                            i0 = max(i_base, 128 * jb)
                            o0 = i0 - i_base
                            ps_s = psa.tile([128, 2, 512], F32, tag="s",
                                            bufs=2, name="ps_s")
                            for hi in range(2):
                                nc.tensor.matmul(
                                    ps_s[:, hi, o0:512],
                                    kT_sb[64 * hi:64 * hi + 64, pair, jb // 4,
                                          (jb % 4) * 128:(jb % 4) * 128 + 128],
                                    qT_sb[64 * hi:64 * hi + 64, pair, icq,
                                          o0:512],
                                    start=True,
                                    stop=True,
                                )
                            et = etpool.tile([128, 2, 512], BF16, tag="et",
                                             name="et")
                            nc.scalar.activation(
                                et[:, :, o0:512], ps_s[:, :, o0:512], EXP,
                                scale=SCALE,
                            )
                            if 128 * jb >= i_base:
                                for hi in range(2):
                                    nc.vector.tensor_tensor(
                                        et[:, hi, o0:o0 + 128],
                                        et[:, hi, o0:o0 + 128],
                                        mask_sb[:], MULT,
                                    )
                            return et, o0

                        def emit_pv(jb, et, o0):
                            for hi in range(2):
                                nc.tensor.matmul(
                                    pos[hi][:, o0:512],
                                    vp_sb[:, jb, heads[hi], :],
                                    et[:, hi, o0:512],
                                    start=(jb == 0),
                                    stop=(jb == jb_last),
                                )

                        pending = emit_s(0)
                        for jb in range(jb_last + 1):
                            nxt = emit_s(jb + 1) if jb < jb_last else None
                            emit_pv(jb, *pending)
                            fill()
                            if nxt is not None:
                                pending = nxt

                        # stage unnormalized O^T and the denominators
                        for hi, h in enumerate(heads):
                            nc.vector.tensor_copy(
                                oT_l[64 * hi:64 * hi + 64, pair, icq, :],
                                pos[hi][0:64, :],
                            )
                            nc.vector.tensor_copy(
                                sums_l[32 * h:32 * h + 1,
                                       i_base:i_base + 512],
                                pos[hi][64:65, :],
                            )
                        fill()

                for u in fit:
                    u()
                return oT_l, sums_l

            # ---- software-pipelined rep loop ----
            # rep r's attention interleaves: rep r-1's normalize + output
            # projection and rep r+1's QKV, one unit per jb iteration.
            xt_cur = xpool.tile([128, 8, T], BF16, tag="xT", bufs=2)
            load_x(xt_cur)
            cur, units0 = qkv_units(xt_cur)
            for u in units0:
                u()
            prev = None
            for _rep in range(nrep):
                if _rep + 1 < nrep:
                    xt_nxt = xpool.tile([128, 8, T], BF16, tag="xT", bufs=2)
                    load_x(xt_nxt)
                fillers = []
                if prev is not None:
                    fillers += norm_units(*prev)
                    fillers += proj_units(prev[0])
                nxt_tiles = None
                if _rep + 1 < nrep:
                    nxt_tiles, qunits = qkv_units(xt_nxt)
                    fillers += qunits
                prev = att_phase(*cur, fillers)
                if nxt_tiles is not None:
                    cur = nxt_tiles
            for u in norm_units(*prev) + proj_units(prev[0]):
                u()

    nc.compile()
    return nc


def _get_nc():
    global _compiled
    if _compiled is None:
        _compiled = _build()
    return _compiled


class _Runner:
    """Compiled PJRT executor for the SPMD kernel, reusable across calls."""

    def __init__(self, nc):
        import jax
        import concourse.mybir as mybir
        from concourse import bass2jax
        from jax.experimental.shard_map import shard_map
        from jax.sharding import Mesh, PartitionSpec

        self.jax = jax
        self.nc = nc
        bass2jax.install_neuronx_cc_hook()

        partition_name = (
            nc.partition_id_tensor.name if nc.partition_id_tensor else None
        )
        in_names, out_names, out_avals, zero_outs = [], [], [], []
        for alloc in nc.m.functions[0].allocations:
            if not isinstance(alloc, mybir.MemoryLocationSet):
                continue
            name = alloc.memorylocations[0].name
            if alloc.kind == "ExternalInput":
                if name != partition_name:
                    in_names.append(name)
            elif alloc.kind == "ExternalOutput":
                out_names.append(name)
                shape = tuple(alloc.tensor_shape)
                dtype = mybir.dt.np(alloc.dtype)
                out_avals.append(jax.core.ShapedArray(shape, dtype))
                zero_outs.append(np.zeros(shape, dtype))
        self.in_names = in_names
        self.out_names = out_names
        self.out_avals = out_avals
        self.zero_outs = zero_outs
        all_names = tuple(in_names + out_names)

        if partition_name is not None:
            all_names = all_names + (partition_name,)

        def _body(*args):
            operands = list(args)
            if partition_name is not None:
                operands.append(bass2jax.partition_id_tensor())
            outs = bass2jax._bass_exec_p.bind(
                *operands,
                out_avals=tuple(out_avals),
                in_names=all_names,
                out_names=tuple(out_names),
                lowering_input_output_aliases=(),
                sim_require_finite=True,
                sim_require_nnan=True,
                nc=nc,
            )
            return tuple(outs)

        devices = jax.devices()[:NCORES]
        assert len(devices) == NCORES
        mesh = Mesh(np.asarray(devices), ("core",))
        self._sharding = jax.sharding.NamedSharding(mesh, PartitionSpec("core"))
        n_args = len(in_names) + len(out_names)
        self.fn = jax.jit(
            shard_map(
                _body,
                mesh=mesh,
                in_specs=(PartitionSpec("core"),) * n_args,
                out_specs=(PartitionSpec("core"),) * len(out_names),
                check_rep=False,
            ),
            keep_unused=True,
        )

    def device_args(self, in_maps):
        args = [
            np.concatenate([np.asarray(m[name]) for m in in_maps], axis=0)
            for name in self.in_names
        ]
        args += [
            np.zeros((NCORES * z.shape[0], *z.shape[1:]), z.dtype)
            for z in self.zero_outs
        ]
        return [self.jax.device_put(a, self._sharding) for a in args]

    def run_device(self, dev_args):
        return self.fn(*dev_args)

    def run(self, in_maps):
        out_arrs = self.fn(*self.device_args(in_maps))
        return [
            {
                name: np.asarray(out_arrs[i]).reshape(
                    NCORES, *self.out_avals[i].shape
                )[c]
                for i, name in enumerate(self.out_names)
            }
            for c in range(NCORES)
        ]


_runner = None


def _get_runner():
    global _runner
    if _runner is None:
        _runner = _Runner(_get_nc())
    return _runner


def make_in_maps(x, Wqkv, Wo):
    import ml_dtypes

    bf16 = ml_dtypes.bfloat16
    x = np.asarray(x, dtype=np.float32)
    Wqkv = np.asarray(Wqkv, dtype=np.float32)
    Wo = np.asarray(Wo, dtype=np.float32)
    mask = np.triu(np.ones((128, 128), dtype=np.float32)).astype(bf16)
    in_maps = []
    for c in range(NCORES):
        b, g = c // 4, c % 4
        in_maps.append({
            "xT": np.ascontiguousarray(x[b].T).astype(bf16),
            "wq": np.ascontiguousarray(
                Wqkv[:, g * CO:(g + 1) * CO]).astype(bf16),
            "wk": np.ascontiguousarray(
                Wqkv[:, C + g * CO:C + (g + 1) * CO]).astype(bf16),
            "wv": np.ascontiguousarray(
                Wqkv[:, 2 * C + g * CO:2 * C + (g + 1) * CO]).astype(bf16),
            "wo": np.ascontiguousarray(Wo[g * CO:(g + 1) * CO, :]).astype(bf16),
            "mask": mask,
        })
    return in_maps


def gather_output(results):
    y = np.zeros((B, T, C), dtype=np.float32)
    for c in range(NCORES):
        y[c // 4] += results[c]["y"]
    return y


def kernel(x, Wqkv, Wo):
    runner = _get_runner()
    in_maps = make_in_maps(x, Wqkv, Wo)
    return gather_output(runner.run(in_maps))
